# revision 5
# baseline (speedup 1.0000x reference)
"""LIF spiking-neuron kernel for Trainium2, data-parallel over 8 NeuronCores.

Reference semantics (T=4, THRESH=1.0, TAU=1.0):
    x: [T*B, N] -> reshape [T, B, N]; mem0 = 0
    per t: mem += x_t; spike_t = (mem >= 1.0); mem *= (1 - spike_t)
    out: spikes reshaped [T*B, N]

Sharding: pure data parallel over B. Core i gets rows i*256:(i+1)*256 of
each timestep block -> shard [T*256, N] = [1024, 4096] f32 in per core.

Spikes are 0.0/1.0, so the kernel stores them as uint8 (4 MiB/core
instead of 16 MiB) and the host casts back to f32 — HBM traffic per core
drops 32 MiB -> 21 MiB. The membrane reset uses one copy_predicated
(mem=0 where spike) instead of is_lt + mult, cutting DVE passes 13 -> 10.

Raw-Bass implementation (the Tile framework's multi-wait instructions don't
pass this container's walrus codegen). Engine split:
  SP (sync, HWDGE)   : all x loads, ring-buffered, prefetch ahead
  ACT (scalar, HWDGE): all spike stores (separate DGE ring so stores
                       waiting on compute never block load prefetch)
  DVE (vector)       : add / is_ge(u8 out) / copy_predicated reset
Per chunk instance [128, N]: mem tile persists across the T=4 recurrence;
t=0 loads x0 directly as mem (no memset, no add); reset skipped at t=3
since mem is dead afterward.
"""

from contextlib import ExitStack

import numpy as np

import concourse.bass as bass
from concourse import mybir
from concourse.bass_utils import run_bass_kernel_spmd

T = 4
B = 2048
N = 4096
N_CORES = 8
BSH = B // N_CORES  # 256 rows per core per timestep
P = 128

F32 = mybir.dt.float32
U8 = mybir.dt.uint8


def build_nc(t_dim=T, bsh=BSH, n=N, bench_iters=None):
    """One-core Bass module: x [t*bsh, n] f32 -> out [t*bsh, n] u8.

    bench_iters: if set, repeat the whole (idempotent) program that many
    times with continuing semaphore counts — used only for slope timing.
    """
    pb = bsh // P  # spatial chunks of [128, n]
    assert bsh % P == 0
    reps = bench_iters or 1
    ng = pb * reps  # chunk instances
    nu = t_dim * ng  # (instance, t) units
    NXB = 5  # x-tile ring
    NSB = 4  # spike-tile ring
    NMEM = 3  # mem rings (one per in-flight chunk)

    nc = bass.Bass()
    x = nc.declare_dram_parameter("x", [t_dim * bsh, n], F32, isOutput=False)
    out = nc.declare_dram_parameter("out", [t_dim * bsh, n], U8, isOutput=True)
    xv = x.rearrange("(t pb p) n -> t pb p n", t=t_dim, pb=pb, p=P)
    ov = out.rearrange("(t pb p) n -> t pb p n", t=t_dim, pb=pb, p=P)

    # --- precompute DVE program order so waits can reference exact counts.
    # v counts DVE instructions (each increments v_sem by 1).
    vidx_ge = {}  # unit u -> v count after its is_ge
    vidx_add = {}  # x-load j -> v count after the add that consumes it
    vidx_last = {}  # instance g -> v count after its final DVE op
    v = 0
    for g in range(ng):
        for t in range(t_dim):
            u = t_dim * g + t
            if t > 0:
                v += 1  # add
                vidx_add[(t_dim - 1) * g + (t - 1)] = v
            v += 1  # is_ge
            vidx_ge[u] = v
            if t < t_dim - 1:
                v += 1  # copy_predicated reset
        vidx_last[g] = v

    with ExitStack() as ctx:
        mem = [
            ctx.enter_context(nc.sbuf_tensor(f"mem{i}", [P, n], F32))
            for i in range(NMEM)
        ]
        zeros = ctx.enter_context(nc.sbuf_tensor("zeros", [P, n], F32))
        xb = [
            ctx.enter_context(nc.sbuf_tensor(f"xb{i}", [P, n], F32))
            for i in range(NXB)
        ]
        sb = [
            ctx.enter_context(nc.sbuf_tensor(f"sb{i}", [P, n], U8))
            for i in range(NSB)
        ]
        # One semaphore per ring slot: concurrent DMA completions interleave
        # their 16 per-engine increments, so a shared cumulative sem cannot
        # identify which DMA finished. Per-slot sems are unambiguous because
        # a slot's next DMA is only issued after its previous reader ran.
        mem_sem = [
            ctx.enter_context(nc.semaphore(f"mem_sem{i}")) for i in range(NMEM)
        ]
        xb_sem = [
            ctx.enter_context(nc.semaphore(f"xb_sem{i}")) for i in range(NXB)
        ]
        sb_sem = [
            ctx.enter_context(nc.semaphore(f"sb_sem{i}")) for i in range(NSB)
        ]
        v_sem = ctx.enter_context(nc.semaphore("v_sem"))
        block = ctx.enter_context(nc.Block())

        @block.sync
        def _(sync):
            for g in range(ng):
                c = g % pb
                if g >= NMEM:  # WAR: mem slot still read by instance g-NMEM
                    sync.wait_ge(v_sem, vidx_last[g - NMEM])
                sync.dma_start(mem[g % NMEM][:], xv[0, c]).then_inc(
                    mem_sem[g % NMEM], 16
                )
                for t in range(1, t_dim):
                    j = (t_dim - 1) * g + (t - 1)
                    if j >= NXB:  # WAR: x slot still read by add j-NXB
                        sync.wait_ge(v_sem, vidx_add[j - NXB])
                    sync.dma_start(xb[j % NXB][:], xv[t, c]).then_inc(
                        xb_sem[j % NXB], 16
                    )

        @block.vector
        def _(vector):
            # DVE is one dependent chain through mem per chunk; wait for all
            # prior DVE ops before each op (same-engine sem waits are
            # already satisfied at issue time, so this costs nothing but
            # guarantees SBUF write visibility across the deep pipeline).
            vector.memset(zeros[:], 0.0)
            v = 0

            def dve(ins):
                nonlocal v
                v += 1
                ins.then_inc(v_sem, 1)

            for g in range(ng):
                m = mem[g % NMEM]
                for t in range(t_dim):
                    u = t_dim * g + t
                    if t == 0:
                        vector.wait_ge(mem_sem[g % NMEM], 16 * (g // NMEM + 1))
                    else:
                        j = (t_dim - 1) * g + (t - 1)
                        vector.wait_ge(xb_sem[j % NXB], 16 * (j // NXB + 1))
                        vector.wait_ge(v_sem, v)
                        dve(vector.tensor_add(m[:], m[:], xb[j % NXB][:]))
                    if u >= NSB:  # WAR: spike slot still being stored
                        vector.wait_ge(sb_sem[u % NSB], 16 * (u // NSB))
                    vector.wait_ge(v_sem, v)
                    dve(
                        vector.tensor_scalar(
                            sb[u % NSB][:], m[:], 1.0, None, mybir.AluOpType.is_ge
                        )
                    )
                    if t < t_dim - 1:
                        vector.wait_ge(v_sem, v)
                        dve(
                            vector.copy_predicated(
                                m[:], sb[u % NSB][:], zeros[:]
                            )
                        )

        @block.scalar
        def _(scalar):
            for u in range(nu):
                g, t = divmod(u, t_dim)
                c = g % pb
                scalar.wait_ge(v_sem, vidx_ge[u])
                scalar.dma_start(ov[t, c], sb[u % NSB][:]).then_inc(
                    sb_sem[u % NSB], 16
                )
            for i in range(NSB):  # drain: all stores landed before NEFF end
                scalar.wait_ge(sb_sem[i], 16 * ((nu - 1 - i) // NSB + 1))

    return nc


def build_nc_bst(t_dim=T, bsh=BSH, n=N, bench_iters=None):
    """Batched-store variant: DVE identical op mix to build_nc, but all
    T*pb spike slices of an iteration land in one [P, T, pb, n] u8 tile,
    written out as a single 4MB DMA per iteration (8x fewer store DMAs,
    and the per-unit store WAR waits leave the DVE stream)."""
    pb = bsh // P
    assert bsh % P == 0
    reps = bench_iters or 1
    ng = pb * reps
    NXB = 4
    NMEM = 3
    NSB = 2  # big store tiles

    nc = bass.Bass()
    x = nc.declare_dram_parameter("x", [t_dim * bsh, n], F32, isOutput=False)
    out = nc.declare_dram_parameter("out", [t_dim * bsh, n], U8, isOutput=True)
    xv = x.rearrange("(t pb p) n -> t pb p n", t=t_dim, pb=pb, p=P)
    ovp = out.rearrange("(t pb p) n -> p t pb n", t=t_dim, pb=pb, p=P)

    vidx_ge = {}
    vidx_add = {}
    vidx_last = {}
    v = 0
    for g in range(ng):
        for t in range(t_dim):
            u = t_dim * g + t
            if t > 0:
                v += 1
                vidx_add[(t_dim - 1) * g + (t - 1)] = v
            v += 1
            vidx_ge[u] = v
            if t < t_dim - 1:
                v += 1
        vidx_last[g] = v

    with ExitStack() as ctx:
        mem = [
            ctx.enter_context(nc.sbuf_tensor(f"mem{i}", [P, n], F32))
            for i in range(NMEM)
        ]
        zeros = ctx.enter_context(nc.sbuf_tensor("zeros", [P, n], F32))
        xb = [
            ctx.enter_context(nc.sbuf_tensor(f"xb{i}", [P, n], F32))
            for i in range(NXB)
        ]
        sbb = [
            ctx.enter_context(nc.sbuf_tensor(f"sbb{i}", [P, t_dim, pb, n], U8))
            for i in range(NSB)
        ]
        mem_sem = [
            ctx.enter_context(nc.semaphore(f"mem_sem{i}")) for i in range(NMEM)
        ]
        xb_sem = [
            ctx.enter_context(nc.semaphore(f"xb_sem{i}")) for i in range(NXB)
        ]
        st_sem = ctx.enter_context(nc.semaphore("st_sem"))
        v_sem = ctx.enter_context(nc.semaphore("v_sem"))
        block = ctx.enter_context(nc.Block())

        @block.sync
        def _(sync):
            for g in range(ng):
                c = g % pb
                if g >= NMEM:
                    sync.wait_ge(v_sem, vidx_last[g - NMEM])
                sync.dma_start(mem[g % NMEM][:], xv[0, c]).then_inc(
                    mem_sem[g % NMEM], 16
                )
                for t in range(1, t_dim):
                    j = (t_dim - 1) * g + (t - 1)
                    if j >= NXB:
                        sync.wait_ge(v_sem, vidx_add[j - NXB])
                    sync.dma_start(xb[j % NXB][:], xv[t, c]).then_inc(
                        xb_sem[j % NXB], 16
                    )

        @block.vector
        def _(vector):
            vector.memset(zeros[:], 0.0)
            v = 0

            def dve(ins):
                nonlocal v
                v += 1
                ins.then_inc(v_sem, 1)

            for g in range(ng):
                i, c = divmod(g, pb)
                m = mem[g % NMEM]
                if c == 0 and i >= NSB:  # WAR: big tile still being stored
                    vector.wait_ge(st_sem, 16 * (i - NSB + 1))
                for t in range(t_dim):
                    if t == 0:
                        vector.wait_ge(mem_sem[g % NMEM], 16 * (g // NMEM + 1))
                    else:
                        j = (t_dim - 1) * g + (t - 1)
                        vector.wait_ge(xb_sem[j % NXB], 16 * (j // NXB + 1))
                        vector.wait_ge(v_sem, v)
                        dve(vector.tensor_add(m[:], m[:], xb[j % NXB][:]))
                    vector.wait_ge(v_sem, v)
                    sl = sbb[i % NSB][:, t, c]
                    dve(
                        vector.tensor_scalar(
                            sl, m[:], 1.0, None, mybir.AluOpType.is_ge
                        )
                    )
                    if t < t_dim - 1:
                        vector.wait_ge(v_sem, v)
                        dve(vector.copy_predicated(m[:], sl, zeros[:]))

        @block.scalar
        def _(scalar):
            for i in range(reps):
                scalar.wait_ge(v_sem, vidx_last[i * pb + pb - 1])
                scalar.dma_start(ovp[:], sbb[i % NSB][:]).then_inc(st_sem, 16)
            scalar.wait_ge(st_sem, 16 * reps)

    return nc


def build_nc_acc(t_dim=T, bsh=BSH, n=N, bench_iters=None):
    """Accum variant: the mem += x_t adds ride the SWDGE loads (CCE add
    during DMA, bit-exact f32), removing all tensor_adds from DVE.
    Stores are batched: all T*pb spike slices of an iteration land in one
    [P, T*pb*n] u8 tile, written out as a single 4MB DMA.

    DVE per chunk: is_ge (u8) x4 + copy_predicated reset x3 = 7 ops.
    """
    pb = bsh // P
    assert bsh % P == 0
    reps = bench_iters or 1
    ng = pb * reps
    NMEM = 2
    NSB = 2  # big store tiles (one per in-flight iteration)

    nc = bass.Bass()
    x = nc.declare_dram_parameter("x", [t_dim * bsh, n], F32, isOutput=False)
    out = nc.declare_dram_parameter("out", [t_dim * bsh, n], U8, isOutput=True)
    xv = x.rearrange("(t pb p) n -> t pb p n", t=t_dim, pb=pb, p=P)
    # store-side view: partition-major so one DMA covers the whole shard
    ovp = out.rearrange("(t pb p) n -> p t pb n", t=t_dim, pb=pb, p=P)

    # DVE program: per (g, t): is_ge, then (t < T-1) copy_predicated.
    vidx_ge = {}
    vidx_cp = {}
    vidx_last = {}
    v = 0
    for g in range(ng):
        for t in range(t_dim):
            v += 1  # is_ge
            vidx_ge[(g, t)] = v
            if t < t_dim - 1:
                v += 1  # copy_predicated
                vidx_cp[(g, t)] = v
        vidx_last[g] = v

    with ExitStack() as ctx:
        mem = [
            ctx.enter_context(nc.sbuf_tensor(f"mem{i}", [P, n], F32))
            for i in range(NMEM)
        ]
        zeros = ctx.enter_context(nc.sbuf_tensor("zeros", [P, n], F32))
        sbb = [
            ctx.enter_context(
                nc.sbuf_tensor(f"sbb{i}", [P, t_dim * pb * n], U8)
            )
            for i in range(NSB)
        ]
        mem_sem = [
            ctx.enter_context(nc.semaphore(f"mem_sem{i}")) for i in range(NMEM)
        ]
        st_sem = ctx.enter_context(nc.semaphore("st_sem"))
        v_sem = ctx.enter_context(nc.semaphore("v_sem"))
        block = ctx.enter_context(nc.Block())

        @block.sync
        def _(sync):
            # plain x0 load per instance
            for g in range(ng):
                c = g % pb
                if g >= NMEM:  # WAR: previous occupant fully consumed
                    sync.wait_ge(v_sem, vidx_last[g - NMEM])
                sync.dma_start(mem[g % NMEM][:], xv[0, c]).then_inc(
                    mem_sem[g % NMEM], 16
                )

        @block.gpsimd
        def _(gp):
            # accumulate loads: mem[slot] += x_t, gated on the reset of t-1
            for g in range(ng):
                c = g % pb
                for t in range(1, t_dim):
                    gp.wait_ge(v_sem, vidx_cp[(g, t - 1)])
                    gp.dma_start(
                        mem[g % NMEM][:], xv[t, c], accum_op=mybir.AluOpType.add
                    ).then_inc(mem_sem[g % NMEM], 16)

        @block.vector
        def _(vector):
            vector.memset(zeros[:], 0.0)
            v = 0

            def dve(ins):
                nonlocal v
                v += 1
                ins.then_inc(v_sem, 1)

            for g in range(ng):
                i, c = divmod(g, pb)
                m = mem[g % NMEM]
                if c == 0 and i >= NSB:  # WAR: big tile still being stored
                    vector.wait_ge(st_sem, 16 * (i - NSB + 1))
                for t in range(t_dim):
                    # mem slot has had 4*(g//NMEM) + t + 1 DMAs at step t
                    vector.wait_ge(
                        mem_sem[g % NMEM], 16 * (t_dim * (g // NMEM) + t + 1)
                    )
                    vector.wait_ge(v_sem, v)
                    q = t * pb + c
                    dve(
                        vector.tensor_scalar(
                            sbb[i % NSB][:, q * n : (q + 1) * n],
                            m[:],
                            1.0,
                            None,
                            mybir.AluOpType.is_ge,
                        )
                    )
                    if t < t_dim - 1:
                        vector.wait_ge(v_sem, v)
                        dve(
                            vector.copy_predicated(
                                m[:],
                                sbb[i % NSB][:, q * n : (q + 1) * n],
                                zeros[:],
                            )
                        )

        @block.scalar
        def _(scalar):
            for i in range(reps):
                scalar.wait_ge(v_sem, vidx_last[i * pb + pb - 1])
                scalar.dma_start(ovp[:], sbb[i % NSB][:]).then_inc(st_sem, 16)
            scalar.wait_ge(st_sem, 16 * reps)

    return nc


_NC_CACHE = None


def _get_nc():
    global _NC_CACHE
    if _NC_CACHE is None:
        _NC_CACHE = build_nc()
    return _NC_CACHE


def shard_input(x):
    """x [T*B, N] -> list of 8 shards [T*BSH, N], C-contiguous."""
    xs = x.reshape(T, B, N)
    return [
        np.ascontiguousarray(xs[:, i * BSH : (i + 1) * BSH, :]).reshape(T * BSH, N)
        for i in range(N_CORES)
    ]


def unshard_output(results):
    """8 shards [T*BSH, N] (u8 or f32) -> full [T*B, N] f32."""
    out = np.empty((T, B, N), dtype=np.float32)
    for i in range(N_CORES):
        out[:, i * BSH : (i + 1) * BSH, :] = results[i].reshape(T, BSH, N)
    return out.reshape(T * B, N)


def run_sharded(x, trace=False):
    nc = _get_nc()
    in_maps = [{"x": s} for s in shard_input(x)]
    res = run_bass_kernel_spmd(nc, in_maps, list(range(N_CORES)), trace=trace)
    return unshard_output([r["out"] for r in res.results]), res


def kernel(x):
    x = np.asarray(x, dtype=np.float32)
    assert x.shape == (T * B, N)
    out, _ = run_sharded(x, trace=False)
    return out
